# revision 20
# baseline (speedup 1.0000x reference)
"""GQA attention block (B=2, S=2048, H=2048, NH=16, NKV=4, HD=128) on 8 TRN2
NeuronCores.

Sharding: 2 batch groups x 4-way tensor parallel over heads.
Core c = b*4 + l handles batch b, q-heads [4l, 4l+4), kv-head l, and computes
output columns [512l, 512(l+1)) of y[b] after per-head AllGathers of the
context over its 4-core group. The host passes x and all weights
pre-transposed/pre-tiled (pure layout; QKV + Wo additionally bf16) so the
device does no layout work on x/W at all.

v2 changes vs the 504us baseline:
  - All A/v transposes moved off the PE onto the DMA XBAR
    (dma_start_transpose, 16x128 tiles): -131k PE cycles and -131k DVE
    copy cycles per core.
  - Softmax shift is an analytic upper bound m_i = max(cs_i*U+, cs_i*U-)
    + const, with U = device-computed column sums of kT (exact for the
    dominant position-ramp term) and a probabilistic bound for the
    remaining unit-scale terms. Removes the per-slot DVE row-max and the
    serial scores->max->exp chain. exp args stay in [-80, +45].
  - Scores PSUM is split into [128,1024] halves with bufs=3 so exp(slot k)
    overlaps scores(slot k+1) (the old single 4-bank S tile serialized
    PE<->ACT every slot).
  - Phase-1 DMA ordering: first x tile + Wk/Wv before Wq, pos tables on the
    ACT queue so they never starve the x-tile queue.
  - Wo shipped bf16 (was f32), ctxf1 halves pulled during phase 2.

Numerics: f32r scores keep the huge position-bias component (~4.7e3 in
logits) accurate; bf16 only where unit-scale. Measured rel err ~7e-3 vs
the f32 reference.
"""
import numpy as np

import concourse.bass as bass
import concourse.mybir as mybir
from concourse import bacc, tile
from concourse.bass_utils import run_bass_kernel_spmd

import ml_dtypes

F32 = mybir.dt.float32
F32R = mybir.dt.float32r
BF16 = mybir.dt.bfloat16
AF = mybir.ActivationFunctionType
ALU = mybir.AluOpType

B, S, H = 2, 2048, 2048
NH, NKV, HD = 16, 4, 128
TP = 4                      # tensor-parallel group size
QH = NH // TP               # q heads per core (4)
OSL = H // TP               # output cols per core (512)
SCALE = 1.0 / np.sqrt(HD)
NHC = H // 128              # 16 contraction chunks of 128
NIT = S // 128              # 16 i-tiles
NJS = S // 512              # 4 j-slices of 512
NISL = S // 512             # 4 i-slices of 512

# exp-arg shift: args <= +SHIFT always (bound >= true max); typical row-max
# args land in [-70, 0]. KONST (host) = s*(7*sqrt(HD))*Cm + B0 - SHIFT.
# Empirical on the reference inputs: args in [-75.8, +31.7] at SHIFT=55.
SHIFT = 55.0

_CACHED = {}


def _build(mask_mode):
    """mask_mode: 'ones' (analytic exp bias) or 'binary' (per-slot subset max
    + additive -1e9 mask bias)."""
    nc = bacc.Bacc("TRN2", target_bir_lowering=False, debug=False, num_devices=8)

    xt = nc.dram_tensor("xt", [H, S], BF16, kind="ExternalInput")
    wqt = nc.dram_tensor("wqt", [128, NHC, OSL], BF16, kind="ExternalInput")
    wkt = nc.dram_tensor("wkt", [128, NHC, HD], BF16, kind="ExternalInput")
    wvt = nc.dram_tensor("wvt", [128, NHC, HD], BF16, kind="ExternalInput")
    wot = nc.dram_tensor("wot", [128, NHC, OSL], BF16, kind="ExternalInput")
    posq = nc.dram_tensor("posq", [128, S], F32, kind="ExternalInput")
    posk = nc.dram_tensor("posk", [128, S], F32, kind="ExternalInput")
    csT = nc.dram_tensor("csT", [128, NIT], F32, kind="ExternalInput")
    bconst = nc.dram_tensor("bconst", [1, 1], F32, kind="ExternalInput")
    maskb = nc.dram_tensor("maskb", [1, S], F32, kind="ExternalInput")
    out = nc.dram_tensor("out", [S, OSL], F32, kind="ExternalOutput")

    groups = [[0, 1, 2, 3], [4, 5, 6, 7]]

    with tile.TileContext(nc) as tc:
        with (
            tc.tile_pool(name="pers", bufs=1) as pers,
            tc.tile_pool(name="small", bufs=16) as small,
            tc.tile_pool(name="dram", bufs=1, space="DRAM") as dram,
        ):
            # ---------------- persistent tiles ----------------
            qt_sb = pers.tile([128, QH, S], F32R)       # [d, h, i]  4MB
            kt_sb = pers.tile([128, S], F32R)           # [d, j]     1MB
            v_sb = pers.tile([128, NHC, HD], BF16)      # [j, jc, d] 0.5MB
            wo_sb = pers.tile([128, NHC, OSL], BF16)    # 2MB
            ctxf0 = [pers.tile([128, TP, S // 2], BF16, name=f"ctxf0_{a}")
                     for a in range(QH)]
            ctxf1 = [pers.tile([128, TP, S // 2], BF16, name=f"ctxf1_{a}")
                     for a in range(QH)]
            ones1f = pers.tile([1, 128], F32)           # mask matmul lhsT src
            nc.vector.memset(ones1f[:], 1.0)
            ones2r = pers.tile([128, 2], F32R)          # colsum lhsT (2 cols:
            ones2f = pers.tile([128, 2], F32)           # 1-part out is illegal)
            nc.vector.memset(ones2f[:], 1.0)
            nc.vector.tensor_copy(ones2r[:], ones2f[:])
            csT_sb = pers.tile([128, NIT], F32)
            upv = pers.tile([1, NISL], F32)             # per-islice colsum maxes
            umv = pers.tile([1, NISL], F32)             # per-islice colsum mins
            negm_sb = pers.tile([128, NIT], F32)        # -m_hat + SHIFT per i
            nc.scalar.dma_start(csT_sb[:], csT[:])

            # AG bounce buffers (per head, split in i-halves for overlap)
            cin = [[dram.tile([128, S // 2], BF16, name=f"cin{h}_{f}")
                    for f in range(2)] for h in range(QH)]
            gout = [[dram.tile([TP * 128, S // 2], BF16, name=f"gout{h}_{f}")
                     for f in range(2)] for h in range(QH)]

            # ---------------- phase 1: QKV projections ----------------
            with (
                tc.tile_pool(name="p1w", bufs=1) as p1w,
                tc.tile_pool(name="p1x", bufs=5) as p1x,
            ):
                wq_sb = p1w.tile([128, NHC, OSL], BF16)
                wk_sb = p1w.tile([128, NHC, HD], BF16)
                wv_sb = p1w.tile([128, NHC, HD], BF16)
                posq_sb = p1w.tile([128, S], F32)
                posk_sb = p1w.tile([128, S], F32)
                vt_stage = p1w.tile([128, S], BF16)      # vT [d, j] staged

                p1ps_cm = tc.tile_pool(name="p1ps", bufs=1, space="PSUM")
                p1ps = p1ps_cm.__enter__()
                for isl in range(4):
                    i0 = isl * 512
                    qp = [p1ps.tile([128, 512], F32, tag=f"q{o}", name=f"qp{o}",
                                    bufs=2 if o < 1 else 1) for o in range(QH)]
                    kp = p1ps.tile([128, 512], F32, tag="k")
                    vp = p1ps.tile([128, 512], F32, tag="v")
                    kcs = p1ps.tile([2, 512], F32, tag="cs")
                    for hc in range(NHC):
                        xt_t = p1x.tile([128, 512], BF16)
                        nc.sync.dma_start(
                            xt_t[:], xt[hc * 128:(hc + 1) * 128, i0:i0 + 512])
                        if isl == 0 and hc == 0:
                            # weights behind the first x tile; first matmul
                            # (k) gates on only ~0.65MB of loads
                            nc.sync.dma_start(wk_sb[:], wkt[:])
                            nc.sync.dma_start(wv_sb[:], wvt[:])
                            nc.sync.dma_start(wq_sb[:, 0:4, :], wqt[:, 0:4, :])
                            nc.sync.dma_start(wq_sb[:, 4:8, :], wqt[:, 4:8, :])
                            nc.sync.dma_start(wq_sb[:, 8:16, :], wqt[:, 8:16, :])
                        st = hc == 0
                        sp = hc == NHC - 1
                        nc.tensor.matmul(kp[:], wk_sb[:, hc, :], xt_t[:],
                                         start=st, stop=sp)
                        nc.tensor.matmul(vp[:], wv_sb[:, hc, :], xt_t[:],
                                         start=st, stop=sp)
                        for o in range(QH):
                            nc.tensor.matmul(
                                qp[o][:], wq_sb[:, hc, o * 128:(o + 1) * 128],
                                xt_t[:], start=st, stop=sp)
                    if isl == 0:
                        # pos tables on the ACT queue: never starve x tiles
                        nc.scalar.dma_start(posq_sb[:], posq[:])
                        nc.scalar.dma_start(posk_sb[:], posk[:])
                    # epilogues: k first so the colsum can chase it
                    nc.vector.scalar_tensor_tensor(
                        kt_sb[:, i0:i0 + 512], kp[:], 1.0,
                        posk_sb[:, i0:i0 + 512],
                        op0=ALU.mult, op1=ALU.add)
                    nc.tensor.matmul(kcs[:], ones2r[:],
                                     kt_sb[:, i0:i0 + 512], start=True,
                                     stop=True)
                    nc.vector.tensor_reduce(
                        upv[:, isl:isl + 1], kcs[0:1, :],
                        axis=mybir.AxisListType.X, op=ALU.max)
                    nc.vector.tensor_reduce(
                        umv[:, isl:isl + 1], kcs[0:1, :],
                        axis=mybir.AxisListType.X, op=ALU.min)
                    for o in range(QH):
                        nc.vector.scalar_tensor_tensor(
                            qt_sb[:, o, i0:i0 + 512], qp[o][:], SCALE,
                            posq_sb[:, i0:i0 + 512],
                            op0=ALU.mult, op1=ALU.add)
                    nc.scalar.copy(vt_stage[:, i0:i0 + 512], vp[:])

                p1ps_cm.__exit__(None, None, None)
                # v transpose on the DMA XBAR: vT [d, j] -> v [j%128, jc, d]
                nc.scalar.dma_start_transpose(v_sb[:], vt_stage[:])

            # wo early (2MB, overlaps all of phase 2)
            nc.sync.dma_start(wo_sb[:], wot[:])

            # mask bias (binary mode): additive row [1, S] f32r for accum-MM
            if mask_mode == "binary":
                maskb_sb = pers.tile([1, S], F32R)
                ones1r_sb = pers.tile([1, 128], F32R)
                nc.gpsimd.dma_start(maskb_sb[:], maskb[:])
                nc.vector.tensor_copy(ones1r_sb[:], ones1f[:])

            # ---------------- negm: analytic exp bias ----------------
            # m_i >= max_j S_ij, exact for the dominant cs_i * U_j ramp term.
            usb = small.tile([1, 2], F32, name="usb")
            nc.vector.tensor_reduce(usb[:, 0:1], upv[:],
                                    axis=mybir.AxisListType.X, op=ALU.max)
            nc.vector.tensor_reduce(usb[:, 1:2], umv[:],
                                    axis=mybir.AxisListType.X, op=ALU.min)
            # broadcast the two device scalars to all partitions via a DRAM
            # round trip (stride-0 partition APs only exist on the DMA path)
            u_dram = dram.tile([1, 2], F32, name="u_dram")
            ubc = small.tile([128, 2], F32, name="ubc")
            kbc = small.tile([128, 1], F32, name="kbc")
            nc.sync.dma_start(u_dram[:], usb[:])
            nc.sync.dma_start(ubc[:], u_dram[:].to_broadcast((128, 2)))
            nc.sync.dma_start(kbc[:], bconst[:].to_broadcast((128, 1)))
            t1 = small.tile([128, NIT], F32, name="t1")
            t2 = small.tile([128, NIT], F32, name="t2")
            nc.vector.tensor_scalar_mul(t1[:], csT_sb[:], ubc[:, 0:1])
            nc.vector.tensor_scalar_mul(t2[:], csT_sb[:], ubc[:, 1:2])
            nc.vector.tensor_tensor(t1[:], t1[:], t2[:], op=ALU.max)
            # negm = -(max(...) + KONST) = (max + KONST) * -1
            nc.vector.tensor_scalar(negm_sb[:], t1[:], kbc[:, 0:1], -1.0,
                                    op0=ALU.add, op1=ALU.mult)

            # ---------------- phase 2: attention per head ----------------
            with (
                tc.tile_pool(name="p2s", bufs=3, space="PSUM") as p2s,
                tc.tile_pool(name="p2cx", bufs=2, space="PSUM") as p2cx,
                tc.tile_pool(name="p2a", bufs=4) as p2a,
                tc.tile_pool(name="p2at", bufs=2) as p2at,
                tc.tile_pool(name="p2ctx", bufs=2) as p2ctx,
            ):
                ctxT = {}

                def scores_slot(h, isl, it4):
                    """scores + exp + normalize + XBAR transposes for one
                    128-query slot; no PE transposes, no row-max in 'ones'
                    mode."""
                    it = isl * 4 + it4
                    qlhs = qt_sb[:, h, it * 128:(it + 1) * 128]
                    sh = [p2s.tile([128, 1024], F32, name=f"S{h}_{it}_{z}",
                                   tag="S") for z in range(2)]
                    for z in range(2):
                        for w in range(2):
                            # f32r moving data caps at 512 elems per matmul
                            nc.tensor.matmul(
                                sh[z][:, w * 512:(w + 1) * 512], qlhs,
                                kt_sb[:, z * 1024 + w * 512:
                                      z * 1024 + (w + 1) * 512],
                                start=True, stop=(mask_mode != "binary"))
                            if mask_mode == "binary":
                                nc.tensor.matmul(
                                    sh[z][:, w * 512:(w + 1) * 512],
                                    ones1r_sb[:],
                                    maskb_sb[:, z * 1024 + w * 512:
                                             z * 1024 + (w + 1) * 512],
                                    start=False, stop=True,
                                    skip_group_check=True)
                    if mask_mode == "binary":
                        # subset row-max (stride 8) as the exp shift
                        nm = small.tile([128, 1], F32, name=f"nm_{h}_{it}",
                                        tag="nm")
                        nm2 = small.tile([128, 1], F32, name=f"nm2_{h}_{it}",
                                         tag="nm2")
                        nc.vector.tensor_reduce(
                            nm[:], sh[0][:, 0:1024:8],
                            axis=mybir.AxisListType.X, op=ALU.max, negate=True)
                        nc.vector.tensor_reduce(
                            nm2[:], sh[1][:, 0:1024:8],
                            axis=mybir.AxisListType.X, op=ALU.max, negate=True)
                        nc.vector.tensor_tensor(nm[:], nm[:], nm2[:],
                                                op=ALU.min)
                        bias = nm[:]
                    else:
                        bias = negm_sb[:, it:it + 1]
                    A = p2a.tile([128, S], BF16, name=f"A_{h}_{it}", tag="A")
                    s0 = small.tile([128, 1], F32, name=f"s0_{h}_{it}", tag="s0")
                    s1 = small.tile([128, 1], F32, name=f"s1_{h}_{it}", tag="s1")
                    nc.scalar.activation(
                        A[:, 0:1024], sh[0][:], AF.Exp,
                        bias=bias, scale=1.0, accum_out=s0[:])
                    nc.scalar.activation(
                        A[:, 1024:2048], sh[1][:], AF.Exp,
                        bias=bias, scale=1.0, accum_out=s1[:])
                    r = small.tile([128, 1], F32, name=f"r_{h}_{it}", tag="r")
                    nc.vector.tensor_tensor(r[:], s0[:], s1[:], op=ALU.add)
                    # clamp: a deep-tail row whose sums denormalize must not
                    # produce inf on the reciprocal (1e-38 only guards inf;
                    # real sums stay above it)
                    nc.vector.tensor_scalar_max(r[:], r[:], 1e-38)
                    nc.vector.reciprocal(r[:], r[:])
                    nc.vector.tensor_scalar_mul(A[:], A[:], r[:])
                    return A

                def emit_transposes(AT, it4, A):
                    # A [128 i, 2048 j] -> AT[j%128, jc, i-slot] via XBAR
                    nc.sync.dma_start_transpose(
                        AT[:, 0:8, it4 * 128:(it4 + 1) * 128], A[:, 0:1024])
                    nc.sync.dma_start_transpose(
                        AT[:, 8:16, it4 * 128:(it4 + 1) * 128], A[:, 1024:2048])

                def consume_quarter(blk, q):
                    """A@V for jc in [4q, 4q+4) of block blk."""
                    h, isl, AT, ctxp = blk
                    for jc in range(4 * q, 4 * q + 4):
                        nc.tensor.matmul(
                            ctxp[:], v_sb[:, jc, :], AT[:, jc, :],
                            start=(jc == 0), stop=(jc == NHC - 1))

                def finish_block(blk):
                    h, isl, AT, ctxp = blk
                    nc.vector.tensor_copy(
                        ctxT[h][:, isl * 512:(isl + 1) * 512], ctxp[:])
                    if isl % 2 == 1:
                        f = isl // 2
                        nc.sync.dma_start(
                            cin[h][f][:],
                            ctxT[h][:, f * 1024:(f + 1) * 1024])
                        nc.gpsimd.collective_compute(
                            "AllGather", ALU.bypass,
                            ins=[cin[h][f][:].opt()],
                            outs=[gout[h][f][:].opt()],
                            replica_groups=groups)
                        # pull each gathered half on-chip as soon as ready
                        if f == 0:
                            nc.sync.dma_start(
                                ctxf0[h][:],
                                gout[h][0][:].rearrange(
                                    "(lr p) i -> p lr i", p=128))
                        else:
                            nc.scalar.dma_start(
                                ctxf1[h][:],
                                gout[h][1][:].rearrange(
                                    "(lr p) i -> p lr i", p=128))

                prev = None
                for h in range(QH):
                    ctxT[h] = p2ctx.tile([128, S], BF16, name=f"ctxT{h}",
                                         tag="ctxT")
                    for isl in range(NISL):
                        AT = p2at.tile([128, NHC, 512], BF16,
                                       name=f"AT{h}_{isl}", tag="AT")
                        ctxp = p2cx.tile([128, 512], F32, name=f"cx{h}_{isl}",
                                         tag="cx")
                        for it4 in range(4):
                            A = scores_slot(h, isl, it4)
                            emit_transposes(AT, it4, A)
                            if prev is not None:
                                consume_quarter(prev, it4)
                                if it4 == 3:
                                    finish_block(prev)
                        prev = (h, isl, AT, ctxp)
                # drain tail
                for q in range(4):
                    consume_quarter(prev, q)
                finish_block(prev)

            # ---------------- phase 3: output projection ----------------
            with (
                tc.tile_pool(name="p3y", bufs=3, space="PSUM") as p3y,
                tc.tile_pool(name="p3o", bufs=3) as p3o,
            ):
                for it in range(NIT):
                    yp = p3y.tile([128, OSL], F32)
                    for cc in range(NHC):
                        a, lr = cc // 4, cc % 4
                        src = (ctxf0[a][:, lr, it * 128:(it + 1) * 128]
                               if it < 8 else
                               ctxf1[a][:, lr, (it - 8) * 128:(it - 7) * 128])
                        nc.tensor.matmul(
                            yp[:], src, wo_sb[:, cc, :],
                            start=(cc == 0), stop=(cc == NHC - 1))
                    y_sb = p3o.tile([128, OSL], F32)
                    nc.vector.tensor_copy(y_sb[:], yp[:])
                    nc.sync.dma_start(out[it * 128:(it + 1) * 128, :], y_sb[:])

    nc.compile()
    return nc


def _get_nc(mask_mode):
    if mask_mode not in _CACHED:
        _CACHED[mask_mode] = _build(mask_mode)
    return _CACHED[mask_mode]


def _make_in_maps(x, attention_mask, position_ids, Wq, Wk, Wv, Wo, mask_mode):
    x = np.asarray(x, dtype=np.float32)
    assert x.shape == (B, S, H), x.shape
    attention_mask = np.asarray(attention_mask, dtype=np.float32)
    position_ids = np.asarray(position_ids)
    Wq = np.asarray(Wq, dtype=np.float32)
    Wk = np.asarray(Wk, dtype=np.float32)
    Wv = np.asarray(Wv, dtype=np.float32)
    Wo = np.asarray(Wo, dtype=np.float32)

    in_maps = []
    for c in range(8):
        b, l = c // TP, c % TP
        pos = position_ids[b].astype(np.float32) * 0.01
        mid = 0.5 * (pos.max() + pos.min())
        cs_row = (pos / np.sqrt(HD)).astype(np.float32)       # cs_i = s*c_i
        posq_b = np.ascontiguousarray(
            np.broadcast_to(cs_row[None, :], (128, S))).astype(np.float32)
        posk_b = np.ascontiguousarray(
            np.broadcast_to((pos - mid)[None, :], (128, S))).astype(np.float32)
        # csT[p, t] = cs_{t*128+p}; KONST = s*(7*sigma_q0s)*Cm + B0 - SHIFT
        csT_b = np.ascontiguousarray(cs_row.reshape(NIT, 128).T)
        Cm = np.abs(pos - mid).max()
        konst = SCALE * (7.0 * np.sqrt(HD)) * Cm + 10.0 - SHIFT
        bconst_b = np.array([[konst]], dtype=np.float32)

        # Wo columns permuted to the gathered order: block a=h, rank lr ->
        # global head 4*lr + h
        wo_sl = Wo[OSL * l:OSL * (l + 1), :]                       # [512, H]
        cols = [wo_sl[:, (4 * lr + h) * HD:(4 * lr + h + 1) * HD]
                for h in range(QH) for lr in range(TP)]
        wo_perm = np.concatenate(cols, axis=1)                     # [512, H]

        maskb_b = (-1e9 * (1.0 - attention_mask[b]))[None, :].astype(np.float32)

        in_maps.append({
            "xt": np.ascontiguousarray(x[b].T).astype(ml_dtypes.bfloat16),
            "wqt": np.ascontiguousarray(
                Wq[OSL * l:OSL * (l + 1), :].T.reshape(NHC, 128, OSL)
                .transpose(1, 0, 2)).astype(ml_dtypes.bfloat16),
            "wkt": np.ascontiguousarray(
                Wk[HD * l:HD * (l + 1), :].T.reshape(NHC, 128, HD)
                .transpose(1, 0, 2)).astype(ml_dtypes.bfloat16),
            "wvt": np.ascontiguousarray(
                Wv[HD * l:HD * (l + 1), :].T.reshape(NHC, 128, HD)
                .transpose(1, 0, 2)).astype(ml_dtypes.bfloat16),
            "wot": np.ascontiguousarray(
                wo_perm.T.reshape(NHC, 128, OSL)
                .transpose(1, 0, 2)).astype(ml_dtypes.bfloat16),
            "posq": posq_b,
            "posk": posk_b,
            "csT": csT_b,
            "bconst": bconst_b,
            "maskb": np.ascontiguousarray(maskb_b),
        })
    return in_maps


def _run(x, attention_mask, position_ids, Wq, Wk, Wv, Wo, trace=False):
    am = np.asarray(attention_mask, dtype=np.float32)
    if np.all(am == 1.0):
        mask_mode = "ones"
    elif np.all((am == 0.0) | (am == 1.0)):
        mask_mode = "binary"
    else:
        mask_mode = "binary"  # fractional masks unsupported exactly; best effort

    nc = _get_nc(mask_mode)
    in_maps = _make_in_maps(x, attention_mask, position_ids, Wq, Wk, Wv, Wo,
                            mask_mode)
    res = run_bass_kernel_spmd(nc, in_maps, core_ids=list(range(8)),
                               trace=trace)
    y = np.empty((B, S, H), dtype=np.float32)
    for c in range(8):
        b, l = c // TP, c % TP
        y[b][:, OSL * l:OSL * (l + 1)] = res.results[c]["out"]
    return y, res


def kernel(**inputs):
    y, _ = _run(**inputs, trace=False)
    return y


def kernel_profiled(**inputs):
    y, res = _run(**inputs, trace=True)
    return y, res


# revision 27
# speedup vs baseline: 1.0165x; 1.0165x over previous
"""GQA attention block (B=2, S=2048, H=2048, NH=16, NKV=4, HD=128) on 8 TRN2
NeuronCores.

Sharding: 2 batch groups x 4-way tensor parallel over heads.
Core c = b*4 + l handles batch b, q-heads [4l, 4l+4), kv-head l, and computes
output columns [512l, 512(l+1)) of y[b] after per-head AllGathers of the
context over its 4-core group. The host passes x and all weights
pre-transposed/pre-tiled (pure layout; QKV + Wo additionally bf16) so the
device does no layout work on x/W at all.

v2 changes vs the 504us baseline:
  - All A/v transposes moved off the PE onto the DMA XBAR
    (dma_start_transpose, 16x128 tiles): -131k PE cycles and -131k DVE
    copy cycles per core.
  - Softmax shift is an analytic upper bound m_i = max(cs_i*U+, cs_i*U-)
    + const, with U = device-computed column sums of kT (exact for the
    dominant position-ramp term) and a probabilistic bound for the
    remaining unit-scale terms. Removes the per-slot DVE row-max and the
    serial scores->max->exp chain. exp args stay in [-80, +45].
  - Scores PSUM is split into [128,1024] halves with bufs=3 so exp(slot k)
    overlaps scores(slot k+1) (the old single 4-bank S tile serialized
    PE<->ACT every slot).
  - Phase-1 DMA ordering: first x tile + Wk/Wv before Wq, pos tables on the
    ACT queue so they never starve the x-tile queue.
  - Wo shipped bf16 (was f32), ctxf1 halves pulled during phase 2.

Numerics: f32r scores keep the huge position-bias component (~4.7e3 in
logits) accurate; bf16 only where unit-scale. Measured rel err ~7e-3 vs
the f32 reference.
"""
import numpy as np

import concourse.bass as bass
import concourse.mybir as mybir
from concourse import bacc, tile
from concourse.bass_utils import run_bass_kernel_spmd

import ml_dtypes

F32 = mybir.dt.float32
F32R = mybir.dt.float32r
BF16 = mybir.dt.bfloat16
AF = mybir.ActivationFunctionType
ALU = mybir.AluOpType

B, S, H = 2, 2048, 2048
NH, NKV, HD = 16, 4, 128
TP = 4                      # tensor-parallel group size
QH = NH // TP               # q heads per core (4)
OSL = H // TP               # output cols per core (512)
SCALE = 1.0 / np.sqrt(HD)
NHC = H // 128              # 16 contraction chunks of 128
NIT = S // 128              # 16 i-tiles
NJS = S // 512              # 4 j-slices of 512
NISL = S // 512             # 4 i-slices of 512

# exp-arg shift: args <= +SHIFT always (bound >= true max); typical row-max
# args land in [-70, 0]. KONST (host) = s*(7*sqrt(HD))*Cm + B0 - SHIFT.
# Empirical on the reference inputs: args in [-75.8, +31.7] at SHIFT=55.
SHIFT = 55.0

_CACHED = {}


def _build(mask_mode):
    """mask_mode: 'ones' (analytic exp bias) or 'binary' (per-slot subset max
    + additive -1e9 mask bias)."""
    nc = bacc.Bacc("TRN2", target_bir_lowering=False, debug=False, num_devices=8)

    xt = nc.dram_tensor("xt", [H, S], BF16, kind="ExternalInput")
    wqt = nc.dram_tensor("wqt", [128, NHC, OSL], BF16, kind="ExternalInput")
    wkt = nc.dram_tensor("wkt", [128, NHC, HD], BF16, kind="ExternalInput")
    wvt = nc.dram_tensor("wvt", [128, NHC, HD], BF16, kind="ExternalInput")
    wot = nc.dram_tensor("wot", [128, NHC, OSL], BF16, kind="ExternalInput")
    posq = nc.dram_tensor("posq", [128, S], F32, kind="ExternalInput")
    posk = nc.dram_tensor("posk", [128, S], F32, kind="ExternalInput")
    csT = nc.dram_tensor("csT", [128, NIT], F32, kind="ExternalInput")
    bconst = nc.dram_tensor("bconst", [1, 1], F32, kind="ExternalInput")
    maskb = nc.dram_tensor("maskb", [1, S], F32, kind="ExternalInput")
    out = nc.dram_tensor("out", [S, OSL], F32, kind="ExternalOutput")

    groups = [[0, 1, 2, 3], [4, 5, 6, 7]]

    with tile.TileContext(nc) as tc:
        with (
            tc.tile_pool(name="pers", bufs=1) as pers,
            tc.tile_pool(name="small", bufs=16) as small,
            tc.tile_pool(name="dram", bufs=1, space="DRAM") as dram,
        ):
            # ---------------- persistent tiles ----------------
            qt_sb = pers.tile([128, QH, S], F32R)       # [d, h, i]  4MB
            kt_sb = pers.tile([128, S], F32R)           # [d, j]     1MB
            v_sb = pers.tile([128, NHC, HD], BF16)      # [j, jc, d] 0.5MB
            wo_sb = pers.tile([128, NHC, OSL], BF16)    # 2MB
            ctxf0 = [pers.tile([128, TP, S // 2], BF16, name=f"ctxf0_{a}")
                     for a in range(QH)]
            ctxf1 = [pers.tile([128, TP, S // 2], BF16, name=f"ctxf1_{a}")
                     for a in range(QH)]
            ones1f = pers.tile([1, 128], F32)           # mask matmul lhsT src
            nc.vector.memset(ones1f[:], 1.0)
            ones2r = pers.tile([128, 2], F32R)          # colsum lhsT (2 cols:
            ones2f = pers.tile([128, 2], F32)           # 1-part out is illegal)
            nc.vector.memset(ones2f[:], 1.0)
            nc.vector.tensor_copy(ones2r[:], ones2f[:])
            csT_sb = pers.tile([128, NIT], F32)
            upv = pers.tile([1, NISL], F32)             # per-islice colsum maxes
            umv = pers.tile([1, NISL], F32)             # per-islice colsum mins
            negm_sb = pers.tile([128, NIT], F32)        # -m_hat + SHIFT per i
            nc.scalar.dma_start(csT_sb[:], csT[:])

            # AG bounce buffers (per head, split in i-halves for overlap)
            cin = [[dram.tile([128, S // 2], BF16, name=f"cin{h}_{f}")
                    for f in range(2)] for h in range(QH)]
            gout = [[dram.tile([TP * 128, S // 2], BF16, name=f"gout{h}_{f}")
                     for f in range(2)] for h in range(QH)]

            # ---------------- phase 1: QKV projections ----------------
            with (
                tc.tile_pool(name="p1w", bufs=1) as p1w,
                tc.tile_pool(name="p1x", bufs=5) as p1x,
            ):
                wq_sb = p1w.tile([128, NHC, OSL], BF16)
                wk_sb = p1w.tile([128, NHC, HD], BF16)
                wv_sb = p1w.tile([128, NHC, HD], BF16)
                posq_sb = p1w.tile([128, S], F32)
                posk_sb = p1w.tile([128, S], F32)
                vt_stage = p1w.tile([128, S], BF16)      # vT [d, j] staged

                p1ps_cm = tc.tile_pool(name="p1ps", bufs=1, space="PSUM")
                p1ps = p1ps_cm.__enter__()
                for isl in range(4):
                    i0 = isl * 512
                    qp = [p1ps.tile([128, 512], F32, tag=f"q{o}", name=f"qp{o}",
                                    bufs=2 if o < 1 else 1) for o in range(QH)]
                    kp = p1ps.tile([128, 512], F32, tag="k")
                    vp = p1ps.tile([128, 512], F32, tag="v")
                    kcs = p1ps.tile([2, 512], F32, tag="cs")
                    for hc in range(NHC):
                        xt_t = p1x.tile([128, 512], BF16)
                        nc.sync.dma_start(
                            xt_t[:], xt[hc * 128:(hc + 1) * 128, i0:i0 + 512])
                        if isl == 0 and hc == 0:
                            # weights behind the first x tile; first matmul
                            # (k) gates on only ~0.65MB of loads
                            nc.sync.dma_start(wk_sb[:], wkt[:])
                            nc.sync.dma_start(wv_sb[:], wvt[:])
                            nc.sync.dma_start(wq_sb[:, 0:4, :], wqt[:, 0:4, :])
                            nc.sync.dma_start(wq_sb[:, 4:8, :], wqt[:, 4:8, :])
                            nc.sync.dma_start(wq_sb[:, 8:16, :], wqt[:, 8:16, :])
                        st = hc == 0
                        sp = hc == NHC - 1
                        nc.tensor.matmul(kp[:], wk_sb[:, hc, :], xt_t[:],
                                         start=st, stop=sp)
                        nc.tensor.matmul(vp[:], wv_sb[:, hc, :], xt_t[:],
                                         start=st, stop=sp)
                        for o in range(QH):
                            nc.tensor.matmul(
                                qp[o][:], wq_sb[:, hc, o * 128:(o + 1) * 128],
                                xt_t[:], start=st, stop=sp)
                    if isl == 0:
                        # pos tables on the ACT queue: never starve x tiles
                        nc.scalar.dma_start(posq_sb[:], posq[:])
                        nc.scalar.dma_start(posk_sb[:], posk[:])
                    # epilogues: k first so the colsum can chase it
                    nc.vector.scalar_tensor_tensor(
                        kt_sb[:, i0:i0 + 512], kp[:], 1.0,
                        posk_sb[:, i0:i0 + 512],
                        op0=ALU.mult, op1=ALU.add)
                    nc.tensor.matmul(kcs[:], ones2r[:],
                                     kt_sb[:, i0:i0 + 512], start=True,
                                     stop=True)
                    nc.vector.tensor_reduce(
                        upv[:, isl:isl + 1], kcs[0:1, :],
                        axis=mybir.AxisListType.X, op=ALU.max)
                    nc.vector.tensor_reduce(
                        umv[:, isl:isl + 1], kcs[0:1, :],
                        axis=mybir.AxisListType.X, op=ALU.min)
                    for o in range(QH):
                        nc.vector.scalar_tensor_tensor(
                            qt_sb[:, o, i0:i0 + 512], qp[o][:], SCALE,
                            posq_sb[:, i0:i0 + 512],
                            op0=ALU.mult, op1=ALU.add)
                    nc.scalar.copy(vt_stage[:, i0:i0 + 512], vp[:])

                p1ps_cm.__exit__(None, None, None)
                # v transpose on the DMA XBAR: vT [d, j] -> v [j%128, jc, d]
                nc.scalar.dma_start_transpose(v_sb[:], vt_stage[:])

            # wo early (2MB, overlaps all of phase 2; SWDGE queue keeps the
            # SP sequencer free for the XBAR transposes)
            nc.gpsimd.dma_start(wo_sb[:], wot[:])

            # mask bias (binary mode): additive row [1, S] f32r for accum-MM
            if mask_mode == "binary":
                maskb_sb = pers.tile([1, S], F32R)
                ones1r_sb = pers.tile([1, 128], F32R)
                nc.gpsimd.dma_start(maskb_sb[:], maskb[:])
                nc.vector.tensor_copy(ones1r_sb[:], ones1f[:])

            # ---------------- negm: analytic exp bias ----------------
            # m_i >= max_j S_ij, exact for the dominant cs_i * U_j ramp term.
            usb = small.tile([1, 2], F32, name="usb")
            nc.vector.tensor_reduce(usb[:, 0:1], upv[:],
                                    axis=mybir.AxisListType.X, op=ALU.max)
            nc.vector.tensor_reduce(usb[:, 1:2], umv[:],
                                    axis=mybir.AxisListType.X, op=ALU.min)
            # broadcast the two device scalars to all partitions via a DRAM
            # round trip (stride-0 partition APs only exist on the DMA path)
            u_dram = dram.tile([1, 2], F32, name="u_dram")
            ubc = small.tile([128, 2], F32, name="ubc")
            kbc = small.tile([128, 1], F32, name="kbc")
            nc.sync.dma_start(u_dram[:], usb[:])
            nc.sync.dma_start(ubc[:], u_dram[:].to_broadcast((128, 2)))
            nc.sync.dma_start(kbc[:], bconst[:].to_broadcast((128, 1)))
            t1 = small.tile([128, NIT], F32, name="t1")
            t2 = small.tile([128, NIT], F32, name="t2")
            nc.vector.tensor_scalar_mul(t1[:], csT_sb[:], ubc[:, 0:1])
            nc.vector.tensor_scalar_mul(t2[:], csT_sb[:], ubc[:, 1:2])
            nc.vector.tensor_tensor(t1[:], t1[:], t2[:], op=ALU.max)
            # negm = -(max(...) + KONST) = (max + KONST) * -1
            nc.vector.tensor_scalar(negm_sb[:], t1[:], kbc[:, 0:1], -1.0,
                                    op0=ALU.add, op1=ALU.mult)

            # ---------------- phase 2: attention per head ----------------
            with (
                tc.tile_pool(name="p2s", bufs=3, space="PSUM") as p2s,
                tc.tile_pool(name="p2cx", bufs=2, space="PSUM") as p2cx,
                tc.tile_pool(name="p2a", bufs=4) as p2a,
                tc.tile_pool(name="p2at", bufs=3) as p2at,
                tc.tile_pool(name="p2ctx", bufs=2) as p2ctx,
            ):
                ctxT = {}

                def scores_slot(h, isl, it4):
                    """scores + exp + normalize + XBAR transposes for one
                    128-query slot; no PE transposes, no row-max in 'ones'
                    mode."""
                    it = isl * 4 + it4
                    qlhs = qt_sb[:, h, it * 128:(it + 1) * 128]
                    sh = [p2s.tile([128, 1024], F32, name=f"S{h}_{it}_{z}",
                                   tag="S") for z in range(2)]
                    for z in range(2):
                        for w in range(2):
                            # f32r moving data caps at 512 elems per matmul
                            nc.tensor.matmul(
                                sh[z][:, w * 512:(w + 1) * 512], qlhs,
                                kt_sb[:, z * 1024 + w * 512:
                                      z * 1024 + (w + 1) * 512],
                                start=True, stop=(mask_mode != "binary"))
                            if mask_mode == "binary":
                                nc.tensor.matmul(
                                    sh[z][:, w * 512:(w + 1) * 512],
                                    ones1r_sb[:],
                                    maskb_sb[:, z * 1024 + w * 512:
                                             z * 1024 + (w + 1) * 512],
                                    start=False, stop=True,
                                    skip_group_check=True)
                    if mask_mode == "binary":
                        # subset row-max (stride 8) as the exp shift
                        nm = small.tile([128, 1], F32, name=f"nm_{h}_{it}",
                                        tag="nm")
                        nm2 = small.tile([128, 1], F32, name=f"nm2_{h}_{it}",
                                         tag="nm2")
                        nc.vector.tensor_reduce(
                            nm[:], sh[0][:, 0:1024:8],
                            axis=mybir.AxisListType.X, op=ALU.max, negate=True)
                        nc.vector.tensor_reduce(
                            nm2[:], sh[1][:, 0:1024:8],
                            axis=mybir.AxisListType.X, op=ALU.max, negate=True)
                        nc.vector.tensor_tensor(nm[:], nm[:], nm2[:],
                                                op=ALU.min)
                        bias = nm[:]
                    else:
                        bias = negm_sb[:, it:it + 1]
                    A = p2a.tile([128, S], BF16, name=f"A_{h}_{it}", tag="A")
                    s0 = small.tile([128, 1], F32, name=f"s0_{h}_{it}", tag="s0")
                    s1 = small.tile([128, 1], F32, name=f"s1_{h}_{it}", tag="s1")
                    nc.scalar.activation(
                        A[:, 0:1024], sh[0][:], AF.Exp,
                        bias=bias, scale=1.0, accum_out=s0[:])
                    nc.scalar.activation(
                        A[:, 1024:2048], sh[1][:], AF.Exp,
                        bias=bias, scale=1.0, accum_out=s1[:])
                    r = small.tile([128, 1], F32, name=f"r_{h}_{it}", tag="r")
                    nc.vector.tensor_tensor(r[:], s0[:], s1[:], op=ALU.add)
                    # clamp: a deep-tail row whose sums denormalize must not
                    # produce inf on the reciprocal (1e-38 only guards inf;
                    # real sums stay above it)
                    nc.vector.tensor_scalar_max(r[:], r[:], 1e-38)
                    nc.vector.reciprocal(r[:], r[:])
                    nc.vector.tensor_scalar_mul(A[:], A[:], r[:])
                    return A

                def emit_transposes(AT, it4, A):
                    # A [128 i, 2048 j] -> AT[j%128, jc, i-slot] via XBAR.
                    # One call per slot: the SP-side issue cost is
                    # ~570ns + 11ns/tile, so fewer+bigger wins.
                    nc.sync.dma_start_transpose(
                        AT[:, :, it4 * 128:(it4 + 1) * 128], A[:])

                def consume_quarter(blk, q):
                    """A@V for jc in [4q, 4q+4) of block blk."""
                    h, isl, AT, ctxp = blk
                    for jc in range(4 * q, 4 * q + 4):
                        nc.tensor.matmul(
                            ctxp[:], v_sb[:, jc, :], AT[:, jc, :],
                            start=(jc == 0), stop=(jc == NHC - 1))

                def finish_block(blk):
                    h, isl, AT, ctxp = blk
                    nc.vector.tensor_copy(
                        ctxT[h][:, isl * 512:(isl + 1) * 512], ctxp[:])
                    if isl % 2 == 1:
                        f = isl // 2
                        nc.sync.dma_start(
                            cin[h][f][:],
                            ctxT[h][:, f * 1024:(f + 1) * 1024])
                        nc.gpsimd.collective_compute(
                            "AllGather", ALU.bypass,
                            ins=[cin[h][f][:].opt()],
                            outs=[gout[h][f][:].opt()],
                            replica_groups=groups)
                        # pull each gathered half on-chip as soon as ready
                        if f == 0:
                            nc.sync.dma_start(
                                ctxf0[h][:],
                                gout[h][0][:].rearrange(
                                    "(lr p) i -> p lr i", p=128))
                        else:
                            nc.scalar.dma_start(
                                ctxf1[h][:],
                                gout[h][1][:].rearrange(
                                    "(lr p) i -> p lr i", p=128))

                prev = None
                for h in range(QH):
                    ctxT[h] = p2ctx.tile([128, S], BF16, name=f"ctxT{h}",
                                         tag="ctxT")
                    for isl in range(NISL):
                        AT = p2at.tile([128, NHC, 512], BF16,
                                       name=f"AT{h}_{isl}", tag="AT")
                        ctxp = p2cx.tile([128, 512], F32, name=f"cx{h}_{isl}",
                                         tag="cx")
                        for it4 in range(4):
                            A = scores_slot(h, isl, it4)
                            emit_transposes(AT, it4, A)
                            # AV quarters lag one extra slot so the XBAR
                            # transpose of prev's last slot has landed
                            if prev is not None and it4 >= 1:
                                consume_quarter(prev, it4 - 1)
                        if prev is not None:
                            consume_quarter(prev, 3)
                            finish_block(prev)
                        prev = (h, isl, AT, ctxp)
                # drain tail
                for q in range(4):
                    consume_quarter(prev, q)
                finish_block(prev)

            # ---------------- phase 3: output projection ----------------
            with (
                tc.tile_pool(name="p3y", bufs=3, space="PSUM") as p3y,
                tc.tile_pool(name="p3o", bufs=3) as p3o,
            ):
                for it in range(NIT):
                    yp = p3y.tile([128, OSL], F32)
                    for cc in range(NHC):
                        a, lr = cc // 4, cc % 4
                        src = (ctxf0[a][:, lr, it * 128:(it + 1) * 128]
                               if it < 8 else
                               ctxf1[a][:, lr, (it - 8) * 128:(it - 7) * 128])
                        nc.tensor.matmul(
                            yp[:], src, wo_sb[:, cc, :],
                            start=(cc == 0), stop=(cc == NHC - 1))
                    y_sb = p3o.tile([128, OSL], F32)
                    nc.vector.tensor_copy(y_sb[:], yp[:])
                    nc.sync.dma_start(out[it * 128:(it + 1) * 128, :], y_sb[:])

    nc.compile()
    return nc


def _get_nc(mask_mode):
    if mask_mode not in _CACHED:
        _CACHED[mask_mode] = _build(mask_mode)
    return _CACHED[mask_mode]


def _make_in_maps(x, attention_mask, position_ids, Wq, Wk, Wv, Wo, mask_mode):
    x = np.asarray(x, dtype=np.float32)
    assert x.shape == (B, S, H), x.shape
    attention_mask = np.asarray(attention_mask, dtype=np.float32)
    position_ids = np.asarray(position_ids)
    Wq = np.asarray(Wq, dtype=np.float32)
    Wk = np.asarray(Wk, dtype=np.float32)
    Wv = np.asarray(Wv, dtype=np.float32)
    Wo = np.asarray(Wo, dtype=np.float32)

    in_maps = []
    for c in range(8):
        b, l = c // TP, c % TP
        pos = position_ids[b].astype(np.float32) * 0.01
        mid = 0.5 * (pos.max() + pos.min())
        cs_row = (pos / np.sqrt(HD)).astype(np.float32)       # cs_i = s*c_i
        posq_b = np.ascontiguousarray(
            np.broadcast_to(cs_row[None, :], (128, S))).astype(np.float32)
        posk_b = np.ascontiguousarray(
            np.broadcast_to((pos - mid)[None, :], (128, S))).astype(np.float32)
        # csT[p, t] = cs_{t*128+p}; KONST = s*(7*sigma_q0s)*Cm + B0 - SHIFT
        csT_b = np.ascontiguousarray(cs_row.reshape(NIT, 128).T)
        Cm = np.abs(pos - mid).max()
        konst = SCALE * (7.0 * np.sqrt(HD)) * Cm + 10.0 - SHIFT
        bconst_b = np.array([[konst]], dtype=np.float32)

        # Wo columns permuted to the gathered order: block a=h, rank lr ->
        # global head 4*lr + h
        wo_sl = Wo[OSL * l:OSL * (l + 1), :]                       # [512, H]
        cols = [wo_sl[:, (4 * lr + h) * HD:(4 * lr + h + 1) * HD]
                for h in range(QH) for lr in range(TP)]
        wo_perm = np.concatenate(cols, axis=1)                     # [512, H]

        maskb_b = (-1e9 * (1.0 - attention_mask[b]))[None, :].astype(np.float32)

        in_maps.append({
            "xt": np.ascontiguousarray(x[b].T).astype(ml_dtypes.bfloat16),
            "wqt": np.ascontiguousarray(
                Wq[OSL * l:OSL * (l + 1), :].T.reshape(NHC, 128, OSL)
                .transpose(1, 0, 2)).astype(ml_dtypes.bfloat16),
            "wkt": np.ascontiguousarray(
                Wk[HD * l:HD * (l + 1), :].T.reshape(NHC, 128, HD)
                .transpose(1, 0, 2)).astype(ml_dtypes.bfloat16),
            "wvt": np.ascontiguousarray(
                Wv[HD * l:HD * (l + 1), :].T.reshape(NHC, 128, HD)
                .transpose(1, 0, 2)).astype(ml_dtypes.bfloat16),
            "wot": np.ascontiguousarray(
                wo_perm.T.reshape(NHC, 128, OSL)
                .transpose(1, 0, 2)).astype(ml_dtypes.bfloat16),
            "posq": posq_b,
            "posk": posk_b,
            "csT": csT_b,
            "bconst": bconst_b,
            "maskb": np.ascontiguousarray(maskb_b),
        })
    return in_maps


def _run(x, attention_mask, position_ids, Wq, Wk, Wv, Wo, trace=False):
    am = np.asarray(attention_mask, dtype=np.float32)
    if np.all(am == 1.0):
        mask_mode = "ones"
    elif np.all((am == 0.0) | (am == 1.0)):
        mask_mode = "binary"
    else:
        mask_mode = "binary"  # fractional masks unsupported exactly; best effort

    nc = _get_nc(mask_mode)
    in_maps = _make_in_maps(x, attention_mask, position_ids, Wq, Wk, Wv, Wo,
                            mask_mode)
    res = run_bass_kernel_spmd(nc, in_maps, core_ids=list(range(8)),
                               trace=trace)
    y = np.empty((B, S, H), dtype=np.float32)
    for c in range(8):
        b, l = c // TP, c % TP
        y[b][:, OSL * l:OSL * (l + 1)] = res.results[c]["out"]
    return y, res


def kernel(**inputs):
    y, _ = _run(**inputs, trace=False)
    return y


def kernel_profiled(**inputs):
    y, res = _run(**inputs, trace=True)
    return y, res
